# revision 9
# baseline (speedup 1.0000x reference)
"""Correlation-cycle (Chamfer) loss kernel for Trainium2, 8 NeuronCores.

reference:  P[b,i,j] = ||x_i||^2 + ||y_j||^2 - 2 x_i.y_j   (x=corr_pred, y=corr_target)
            out = (mean_{b,j} min_i clip(P,0,100) + mean_{b,i} min_j clip(P,0,100)) / B

Sharding: B=4 batches x 2 i-halves -> 8 cores. Each core owns an x-half
(2048 rows) and the full y (4096 rows) of one batch.

Scheme "hybrid" (default): ONE matmul orientation [i x j]; each PSUM group
[128 x 2048] (= -2*z, bf16 inputs / fp32 accumulate) is consumed by exactly
two fused DVE ops:
  tensor_tensor_reduce: u_bf16 = psum + y2bc ( = y2_j - 2 z_ij );
                        rowacc[:,ic] = min(rowacc[:,ic], min_j u)   (row mins)
  scalar_tensor_tensor: colacc = min(colacc, u + x2_i)             (col mins,
                        colacc accumulates the full P over i-chunks per lane)
Host: min over lanes/cores (+x2_i for rows), clip(0,100) -- clip commutes
with min -- then means.  No ACT/Pool usage; minimal instruction count (the
axon-virtualized NeuronCores are instruction-dispatch-bound at ~2us/inst,
so half the matmuls + 2 DVE ops per group beat any multi-engine split).

Scheme "pf": two orientations with a D/A engine split (kept for A/B).
"""

import numpy as np
import ml_dtypes

import concourse.bass as bass
import concourse.mybir as mybir
import concourse.tile as tile
from concourse import bacc
from concourse.bass_utils import run_bass_kernel_spmd

BF16 = ml_dtypes.bfloat16
F32 = np.float32

B, N, D = 4, 4096, 128
NCORES = 8
NI = N // 2          # per-core i range (half a batch)
NJ = N               # full j range
GW = 2048            # psum group width (4 banks)
MMW = 512            # matmul moving width (1 bank)
BIG = 1.0e38         # accumulator init (min identity; fits bf16)

AluOp = mybir.AluOpType
ActFn = mybir.ActivationFunctionType

# pf-scheme routing pattern (D = DVE-direct fp32, A = ACT->DVE bf16)
PAT1 = ['D', 'A', 'A', 'A'] * 4
PAT2 = PAT1 + PAT1


def build(ni=NI, nj=NJ, gw=GW, reps=1, scheme="hybrid"):
    if scheme == "pf":
        return build_pf(ni, nj, min(gw, 1024), reps)
    if scheme == "v2":
        return build_v2(ni, nj, gw, reps)
    if scheme == "v4":
        return build_v4(ni, nj, min(gw, 2048), reps)
    n_ic = ni // 128
    n_jg = nj // gw

    nc = bacc.Bacc("TRN2", target_bir_lowering=False, debug=False,
                   enable_asserts=False, num_devices=NCORES)
    f32 = mybir.dt.float32
    bf16 = mybir.dt.bfloat16

    xT_d = nc.dram_tensor("xT", [128, ni], bf16, kind="ExternalInput")
    m2yT_d = nc.dram_tensor("m2yT", [128, nj], bf16, kind="ExternalInput")
    x2c_d = nc.dram_tensor("x2c", [128, n_ic], f32, kind="ExternalInput")
    y2bc_d = nc.dram_tensor("y2bc", [128, nj], bf16, kind="ExternalInput")
    colB_d = nc.dram_tensor("colB", [128, nj], bf16, kind="ExternalOutput")
    rowR_d = nc.dram_tensor("rowR", [128, n_ic * n_jg], f32, kind="ExternalOutput")

    with tile.TileContext(nc) as tc:
        with (
            tc.tile_pool(name="persist", bufs=1) as persist,
            tc.tile_pool(name="psum", bufs=2, space="PSUM") as psum_pool,
            tc.tile_pool(name="u", bufs=3) as upool,
        ):
            xT = persist.tile([128, ni], bf16, name="xT")
            m2yT = persist.tile([128, nj], bf16, name="m2yT")
            x2c = persist.tile([128, n_ic], f32, name="x2c")
            y2bc = persist.tile([128, nj], bf16, name="y2bc")
            colB = persist.tile([128, nj], bf16, name="colB")
            rowR = persist.tile([128, n_ic * n_jg], f32, name="rowR")

            nc.sync.dma_start(out=xT[:, :], in_=xT_d[:, :])
            ck = min(2048, nj)
            for c0 in range(0, nj, ck):
                nc.sync.dma_start(out=m2yT[:, c0:c0 + ck], in_=m2yT_d[:, c0:c0 + ck])
                nc.sync.dma_start(out=y2bc[:, c0:c0 + ck], in_=y2bc_d[:, c0:c0 + ck])
            nc.sync.dma_start(out=x2c[:, :], in_=x2c_d[:, :])
            nc.vector.memset(colB[:, :], BIG)

            def emit_body():
                for ic in range(n_ic):
                    for jg in range(n_jg):
                        sl = slice(jg * gw, (jg + 1) * gw)
                        psum = psum_pool.tile([128, gw], f32, tag="ps", name="ps")
                        for q in range(gw // MMW):
                            j0 = jg * gw + q * MMW
                            nc.tensor.matmul(
                                psum[:, q * MMW:(q + 1) * MMW],
                                xT[:, ic * 128:(ic + 1) * 128],
                                m2yT[:, j0:j0 + MMW])
                        u = upool.tile([128, gw], bf16, tag="u", name="u")
                        nc.vector.tensor_tensor(
                            u[:, :], psum[:, :], y2bc[:, sl], AluOp.add)
                        k = ic * n_jg + jg
                        nc.vector.tensor_reduce(
                            rowR[:, k:k + 1], u[:, :],
                            mybir.AxisListType.X, AluOp.min)
                        nc.vector.scalar_tensor_tensor(
                            colB[:, sl], u[:, :], x2c[:, ic:ic + 1],
                            colB[:, sl], AluOp.add, AluOp.min)

            if reps > 1:
                with tc.For_i(0, reps, 1,
                              hint_engines=(mybir.EngineType.PE,
                                            mybir.EngineType.DVE)):
                    emit_body()
            else:
                emit_body()

            for c0 in range(0, nj, ck):
                nc.sync.dma_start(out=colB_d[:, c0:c0 + ck], in_=colB[:, c0:c0 + ck])
            nc.sync.dma_start(out=rowR_d[:, :], in_=rowR[:, :])

    nc.compile()
    return nc


def build_v2(ni=NI, nj=NJ, gw=2048, reps=1):
    """Fused scheme: per [128 x gw] psum group exactly TWO DVE ops.

    tensor_tensor_reduce: u = psum + y2bc (bf16, dead store);
                          rowR[:, k] = min_j u            (row path)
    scalar_tensor_tensor: colB = min(colB, psum + x2_i)   (col path;
                          y2_j commutes with min over i -> host adds it)
    """
    n_ic = ni // 128
    n_jg = nj // gw
    psum_bufs = 2 if gw <= 2048 else 1

    nc = bacc.Bacc("TRN2", target_bir_lowering=False, debug=False,
                   enable_asserts=False, num_devices=NCORES)
    f32 = mybir.dt.float32
    bf16 = mybir.dt.bfloat16

    xT_d = nc.dram_tensor("xT", [128, ni], bf16, kind="ExternalInput")
    m2yT_d = nc.dram_tensor("m2yT", [128, nj], bf16, kind="ExternalInput")
    x2c_d = nc.dram_tensor("x2c", [128, n_ic], f32, kind="ExternalInput")
    y2bc_d = nc.dram_tensor("y2bc", [128, nj], bf16, kind="ExternalInput")
    colB_d = nc.dram_tensor("colB", [128, nj], f32, kind="ExternalOutput")
    rowR_d = nc.dram_tensor("rowR", [128, n_ic * n_jg], f32, kind="ExternalOutput")

    with tile.TileContext(nc) as tc:
        with (
            tc.tile_pool(name="persist", bufs=1) as persist,
            tc.tile_pool(name="psum", bufs=psum_bufs, space="PSUM") as psum_pool,
            tc.tile_pool(name="u", bufs=3) as upool,
        ):
            xT = persist.tile([128, ni], bf16, name="xT")
            m2yT = persist.tile([128, nj], bf16, name="m2yT")
            x2c = persist.tile([128, n_ic], f32, name="x2c")
            y2bc = persist.tile([128, nj], bf16, name="y2bc")
            colB = persist.tile([128, nj], f32, name="colB")
            rowR = persist.tile([128, n_ic * n_jg], f32, name="rowR")

            nc.sync.dma_start(out=xT[:, :], in_=xT_d[:, :])
            ck = min(2048, nj)
            for c0 in range(0, nj, ck):
                nc.sync.dma_start(out=m2yT[:, c0:c0 + ck], in_=m2yT_d[:, c0:c0 + ck])
                nc.sync.dma_start(out=y2bc[:, c0:c0 + ck], in_=y2bc_d[:, c0:c0 + ck])
            nc.sync.dma_start(out=x2c[:, :], in_=x2c_d[:, :])
            nc.vector.memset(colB[:, :], BIG)

            def emit_body():
                for ic in range(n_ic):
                    for jg in range(n_jg):
                        sl = slice(jg * gw, (jg + 1) * gw)
                        psum = psum_pool.tile([128, gw], f32, tag="ps", name="ps")
                        for q in range(gw // MMW):
                            j0 = jg * gw + q * MMW
                            nc.tensor.matmul(
                                psum[:, q * MMW:(q + 1) * MMW],
                                xT[:, ic * 128:(ic + 1) * 128],
                                m2yT[:, j0:j0 + MMW])
                        u = upool.tile([128, gw], bf16, tag="u", name="u")
                        k = ic * n_jg + jg
                        nc.vector.tensor_tensor_reduce(
                            out=u[:, :], in0=psum[:, :], in1=y2bc[:, sl],
                            scale=1.0, scalar=BIG,
                            op0=AluOp.add, op1=AluOp.min,
                            accum_out=rowR[:, k:k + 1])
                        nc.vector.scalar_tensor_tensor(
                            colB[:, sl], psum[:, :], x2c[:, ic:ic + 1],
                            colB[:, sl], AluOp.add, AluOp.min)

            if reps > 1:
                with tc.For_i(0, reps, 1,
                              hint_engines=(mybir.EngineType.PE,
                                            mybir.EngineType.DVE)):
                    emit_body()
            else:
                emit_body()

            for c0 in range(0, nj, ck):
                nc.sync.dma_start(out=colB_d[:, c0:c0 + ck], in_=colB[:, c0:c0 + ck])
            nc.sync.dma_start(out=rowR_d[:, :], in_=rowR[:, :])

    nc.compile()
    return nc


def build_v4(ni=NI, nj=NJ, gw=2048, reps=1):
    """3-engine pipeline, minimal DVE work.

    PE:  psum = y2_j - 2 x_i.y_j   (main matmul + K=1 ones-row accumulate
         matmul that broadcasts y2 along partitions)
    ACT: u16 = bf16(psum + x2_i)   (per-partition bias; u16 = full P)
    DVE: rowR[:, ic] = min_j u16   (tensor_reduce, FD = nj)
         colB = min(colB, u16)     (tensor_tensor min, bf16 2x)
    Host: clips + means; no bias corrections needed (P is complete).
    """
    n_ic = ni // 128
    n_jg = nj // gw

    nc = bacc.Bacc("TRN2", target_bir_lowering=False, debug=False,
                   enable_asserts=False, num_devices=NCORES)
    f32 = mybir.dt.float32
    bf16 = mybir.dt.bfloat16

    xT_d = nc.dram_tensor("xT", [128, ni], bf16, kind="ExternalInput")
    m2yT_d = nc.dram_tensor("m2yT", [128, nj], bf16, kind="ExternalInput")
    x2c_d = nc.dram_tensor("x2c", [128, n_ic], f32, kind="ExternalInput")
    y2r_d = nc.dram_tensor("y2r", [1, nj], bf16, kind="ExternalInput")
    colB_d = nc.dram_tensor("colB", [128, nj], bf16, kind="ExternalOutput")
    rowR_d = nc.dram_tensor("rowR", [128, n_ic], f32, kind="ExternalOutput")

    with tile.TileContext(nc) as tc:
        with (
            tc.tile_pool(name="persist", bufs=1) as persist,
            tc.tile_pool(name="psum", bufs=2, space="PSUM") as psum_pool,
            tc.tile_pool(name="u", bufs=2) as upool,
        ):
            xT = persist.tile([128, ni], bf16, name="xT")
            m2yT = persist.tile([128, nj], bf16, name="m2yT")
            x2c = persist.tile([128, n_ic], f32, name="x2c")
            y2r = persist.tile([1, nj], bf16, name="y2r")
            ones = persist.tile([1, 128], bf16, name="ones")
            colB = persist.tile([128, nj], bf16, name="colB")
            rowR = persist.tile([128, n_ic], f32, name="rowR")

            nc.sync.dma_start(out=xT[:, :], in_=xT_d[:, :])
            nc.sync.dma_start(out=m2yT[:, :], in_=m2yT_d[:, :])
            nc.sync.dma_start(out=x2c[:, :], in_=x2c_d[:, :])
            nc.sync.dma_start(out=y2r[:, :], in_=y2r_d[:, :])
            nc.vector.memset(ones[:, :], 1.0)
            nc.vector.memset(colB[:, :], BIG)

            def emit_body():
                for ic in range(n_ic):
                    u16 = upool.tile([128, nj], bf16, tag="u", name="u")
                    for jg in range(n_jg):
                        sl = slice(jg * gw, (jg + 1) * gw)
                        psum = psum_pool.tile([128, gw], f32, tag="ps",
                                              name="ps")
                        for q in range(gw // MMW):
                            j0 = jg * gw + q * MMW
                            qs = slice(q * MMW, (q + 1) * MMW)
                            nc.tensor.matmul(
                                psum[:, qs],
                                xT[:, ic * 128:(ic + 1) * 128],
                                m2yT[:, j0:j0 + MMW],
                                start=True, stop=False)
                            nc.tensor.matmul(
                                psum[:, qs], ones[:, :],
                                y2r[:, j0:j0 + MMW],
                                start=False, stop=True)
                        nc.scalar.activation(
                            u16[:, sl], psum[:, :], ActFn.Identity,
                            bias=x2c[:, ic:ic + 1], scale=1.0)
                    nc.vector.tensor_reduce(
                        rowR[:, ic:ic + 1], u16[:, :],
                        mybir.AxisListType.X, AluOp.min)
                    nc.vector.tensor_tensor(
                        colB[:, :], u16[:, :], colB[:, :], AluOp.min)

            if reps > 1:
                with tc.For_i(0, reps, 1,
                              hint_engines=(mybir.EngineType.PE,
                                            mybir.EngineType.DVE,
                                            mybir.EngineType.Activation)):
                    emit_body()
            else:
                emit_body()

            nc.sync.dma_start(out=colB_d[:, :], in_=colB[:, :])
            nc.sync.dma_start(out=rowR_d[:, :], in_=rowR[:, :])

    nc.compile()
    return nc


def build_pf(ni, nj, gw, reps):
    """Two-orientation scheme with DVE/ACT split (fallback / A-B testing)."""
    n_ic = ni // 128
    n_jc = nj // 128
    pat1 = PAT1[:n_ic]
    pat2 = PAT2[:n_jc]
    paths = set(pat1) | set(pat2)

    nc = bacc.Bacc("TRN2", target_bir_lowering=False, debug=False,
                   enable_asserts=False, num_devices=NCORES)
    f32 = mybir.dt.float32
    bf16 = mybir.dt.bfloat16

    xT_d = nc.dram_tensor("xT", [128, ni], bf16, kind="ExternalInput")
    m2yT_d = nc.dram_tensor("m2yT", [128, nj], bf16, kind="ExternalInput")
    x2c_d = nc.dram_tensor("x2c", [128, n_ic], f32, kind="ExternalInput")
    y2c_d = nc.dram_tensor("y2c", [128, n_jc], f32, kind="ExternalInput")
    col_d, row_d = {}, {}
    for p in sorted(paths):
        dt = f32 if p == 'D' else bf16
        col_d[p] = nc.dram_tensor("col" + p, [128, nj], dt, kind="ExternalOutput")
        row_d[p] = nc.dram_tensor("row" + p, [128, ni], dt, kind="ExternalOutput")

    with tile.TileContext(nc) as tc:
        with (
            tc.tile_pool(name="persist", bufs=1) as persist,
            tc.tile_pool(name="psum", bufs=4, space="PSUM") as psum_pool,
            tc.tile_pool(name="u", bufs=6) as upool,
        ):
            xT = persist.tile([128, ni], bf16, name="xT")
            m2yT = persist.tile([128, nj], bf16, name="m2yT")
            x2c = persist.tile([128, n_ic], f32, name="x2c")
            y2c = persist.tile([128, n_jc], f32, name="y2c")
            col_s = {p: persist.tile([128, nj], f32 if p == 'D' else bf16,
                                     name="col" + p, tag="col" + p)
                     for p in sorted(paths)}
            row_s = {p: persist.tile([128, ni], f32 if p == 'D' else bf16,
                                     name="row" + p, tag="row" + p)
                     for p in sorted(paths)}

            ck = min(1024, ni, nj)
            for c0 in range(0, ni, ck):
                nc.sync.dma_start(out=xT[:, c0:c0 + ck], in_=xT_d[:, c0:c0 + ck])
            for c0 in range(0, nj, ck):
                nc.sync.dma_start(out=m2yT[:, c0:c0 + ck], in_=m2yT_d[:, c0:c0 + ck])
            nc.sync.dma_start(out=x2c[:, :], in_=x2c_d[:, :])
            nc.sync.dma_start(out=y2c[:, :], in_=y2c_d[:, :])

            def consume(path, psum, bias, accs, sl, first):
                acc = accs[path]
                if path == 'D':
                    if first:
                        nc.vector.tensor_scalar(
                            acc[:, sl], psum[:, :], bias, None, AluOp.add)
                    else:
                        nc.vector.scalar_tensor_tensor(
                            acc[:, sl], psum[:, :], bias, acc[:, sl],
                            AluOp.add, AluOp.min)
                    return
                u = upool.tile([128, psum.shape[1]], bf16, name="u", tag="u")
                nc.scalar.activation(u[:, :], psum[:, :], ActFn.Identity,
                                     bias=bias, scale=1.0)
                if first:
                    nc.vector.tensor_copy(acc[:, sl], u[:, :])
                else:
                    nc.vector.tensor_tensor(acc[:, sl], u[:, :], acc[:, sl],
                                            AluOp.min)

            def emit_body():
                for jg in range(nj // gw):
                    sl = slice(jg * gw, (jg + 1) * gw)
                    seen = set()
                    for ic in range(n_ic):
                        path = pat1[ic]
                        psum = psum_pool.tile([128, gw], f32, tag="ps", name="ps")
                        for q in range(gw // MMW):
                            j0 = jg * gw + q * MMW
                            nc.tensor.matmul(
                                psum[:, q * MMW:(q + 1) * MMW],
                                xT[:, ic * 128:(ic + 1) * 128],
                                m2yT[:, j0:j0 + MMW])
                        consume(path, psum, x2c[:, ic:ic + 1], col_s, sl,
                                path not in seen)
                        seen.add(path)
                gw2 = min(gw, ni)
                for ig in range(ni // gw2):
                    sl = slice(ig * gw2, (ig + 1) * gw2)
                    seen = set()
                    for jc in range(n_jc):
                        path = pat2[jc]
                        psum = psum_pool.tile([128, gw2], f32, tag="ps", name="ps")
                        for q in range(gw2 // MMW):
                            i0 = ig * gw2 + q * MMW
                            nc.tensor.matmul(
                                psum[:, q * MMW:(q + 1) * MMW],
                                m2yT[:, jc * 128:(jc + 1) * 128],
                                xT[:, i0:i0 + MMW])
                        consume(path, psum, y2c[:, jc:jc + 1], row_s, sl,
                                path not in seen)
                        seen.add(path)

            if reps > 1:
                with tc.For_i(0, reps, 1,
                              hint_engines=(mybir.EngineType.PE,
                                            mybir.EngineType.DVE,
                                            mybir.EngineType.Activation)):
                    emit_body()
            else:
                emit_body()

            for p in sorted(paths):
                nc.sync.dma_start(out=col_d[p][:, :], in_=col_s[p][:, :])
                nc.sync.dma_start(out=row_d[p][:, :], in_=row_s[p][:, :])

    nc.compile()
    return nc


def host_prep(x, y, scheme="hybrid"):
    """Per-core input maps. Core c: batch c//2, i-half c%2."""
    x = np.ascontiguousarray(np.asarray(x, F32))
    y = np.ascontiguousarray(np.asarray(y, F32))
    x16 = x.astype(BF16)
    y16 = y.astype(BF16)
    m2y16 = (y16.astype(F32) * -2.0).astype(BF16)          # exact in bf16
    x2 = (x16.astype(F32) ** 2).sum(-1)                    # [B, N]
    y2 = (y16.astype(F32) ** 2).sum(-1)
    in_maps = []
    for c in range(NCORES):
        b, h = divmod(c, 2)
        i0 = h * NI
        m = {
            "xT": np.ascontiguousarray(x16[b, i0:i0 + NI, :].T),
            "m2yT": np.ascontiguousarray(m2y16[b].T),
            "x2c": np.ascontiguousarray(x2[b, i0:i0 + NI].reshape(NI // 128, 128).T),
        }
        if scheme == "v4":
            m["y2r"] = np.ascontiguousarray(y2[b].astype(BF16)[None, :])
        elif scheme in ("hybrid", "v2"):
            m["y2bc"] = np.ascontiguousarray(
                np.broadcast_to(y2[b].astype(BF16), (128, N)))
        else:
            m["y2c"] = np.ascontiguousarray(y2[b].reshape(N // 128, 128).T)
        in_maps.append(m)
    return in_maps, x2, y2


def combine(results, x2, y2, scheme="hybrid"):
    col_mins = np.empty((B, N), F32)
    row_mins = np.empty((B, N), F32)
    for b in range(B):
        cores = [results[2 * b], results[2 * b + 1]]
        if scheme == "v4":
            col = np.minimum.reduce(
                [r["colB"].astype(F32).min(0) for r in cores])
            col_mins[b] = np.clip(col, 0.0, 100.0)
            for h, r in enumerate(cores):
                row = r["rowR"].T.reshape(-1)          # [NI], i = ic*128+lane
                i0 = h * NI
                row_mins[b, i0:i0 + NI] = np.clip(row, 0.0, 100.0)
        elif scheme == "v2":
            col = np.minimum.reduce([r["colB"].min(0) for r in cores])
            col_mins[b] = np.clip(col + y2[b], 0.0, 100.0)
            for h, r in enumerate(cores):
                rr = r["rowR"]                         # [128, n_ic*n_jg]
                n_jg = rr.shape[1] // (NI // 128)
                rr = rr.reshape(128, NI // 128, n_jg).min(axis=2)
                row = rr.T.reshape(-1)                 # [NI], i = ic*128 + lane
                i0 = h * NI
                row_mins[b, i0:i0 + NI] = np.clip(
                    row + x2[b, i0:i0 + NI], 0.0, 100.0)
        elif scheme == "hybrid":
            col = np.minimum.reduce([r["colB"].astype(F32).min(0) for r in cores])
            col_mins[b] = np.clip(col, 0.0, 100.0)
            for h, r in enumerate(cores):
                rr = r["rowR"]                         # [128, n_ic*n_jg]
                n_jg = N // GW
                rr = rr.reshape(128, NI // 128, n_jg).min(axis=2)
                row = rr.T.reshape(-1)                 # [NI], i = ic*128 + lane
                i0 = h * NI
                row_mins[b, i0:i0 + NI] = np.clip(
                    row + x2[b, i0:i0 + NI], 0.0, 100.0)
        else:
            col = np.minimum.reduce([
                np.minimum.reduce([r[k].astype(F32).min(0)
                                   for k in r if k.startswith("col")])
                for r in cores])
            col_mins[b] = np.clip(col + y2[b], 0.0, 100.0)
            for h, r in enumerate(cores):
                row = np.minimum.reduce([r[k].astype(F32).min(0)
                                         for k in r if k.startswith("row")])
                i0 = h * NI
                row_mins[b, i0:i0 + NI] = np.clip(
                    row + x2[b, i0:i0 + NI], 0.0, 100.0)
    out = (col_mins.mean(dtype=np.float64) + row_mins.mean(dtype=np.float64)) / B
    return np.asarray(out, dtype=F32)


_CACHE = {}
TRACE = False
LAST_RESULTS = None
SCHEME = "hybrid"


def kernel(corr_pred, corr_target):
    global LAST_RESULTS
    key = ("nc", SCHEME)
    if key not in _CACHE:
        _CACHE[key] = build(scheme=SCHEME)
    nc = _CACHE[key]
    in_maps, x2, y2 = host_prep(corr_pred, corr_target, scheme=SCHEME)
    res = run_bass_kernel_spmd(nc, in_maps, core_ids=list(range(NCORES)),
                               trace=TRACE)
    LAST_RESULTS = res
    return combine(res.results, x2, y2, scheme=SCHEME)



# revision 17
# speedup vs baseline: 1.7361x; 1.7361x over previous
"""Correlation-cycle (Chamfer) loss kernel for Trainium2, 8 NeuronCores.

reference:  P[b,i,j] = ||x_i||^2 + ||y_j||^2 - 2 x_i.y_j   (x=corr_pred, y=corr_target)
            out = (mean_{b,j} min_i clip(P,0,100) + mean_{b,i} min_j clip(P,0,100)) / B

Sharding: B=4 batches x 2 i-halves -> 8 cores. Each core owns an x-half
(2048 rows) and the full y (4096 rows) of one batch.

Scheme "hybrid" (default): ONE matmul orientation [i x j]; each PSUM group
[128 x 2048] (= -2*z, bf16 inputs / fp32 accumulate) is consumed by exactly
two fused DVE ops:
  tensor_tensor_reduce: u_bf16 = psum + y2bc ( = y2_j - 2 z_ij );
                        rowacc[:,ic] = min(rowacc[:,ic], min_j u)   (row mins)
  scalar_tensor_tensor: colacc = min(colacc, u + x2_i)             (col mins,
                        colacc accumulates the full P over i-chunks per lane)
Host: min over lanes/cores (+x2_i for rows), clip(0,100) -- clip commutes
with min -- then means.  No ACT/Pool usage; minimal instruction count (the
axon-virtualized NeuronCores are instruction-dispatch-bound at ~2us/inst,
so half the matmuls + 2 DVE ops per group beat any multi-engine split).

Scheme "pf": two orientations with a D/A engine split (kept for A/B).
"""

import numpy as np
import ml_dtypes

import concourse.bass as bass
import concourse.mybir as mybir
import concourse.tile as tile
from concourse import bacc
from concourse.bass_utils import run_bass_kernel_spmd

BF16 = ml_dtypes.bfloat16
F32 = np.float32

B, N, D = 4, 4096, 128
NCORES = 8
NI = N // 2          # per-core i range (half a batch)
NJ = N               # full j range
GW = 2048            # psum group width (4 banks)
MMW = 512            # matmul moving width (1 bank)
BIG = 1.0e38         # accumulator init (min identity; fits bf16)

AluOp = mybir.AluOpType
ActFn = mybir.ActivationFunctionType

# pf-scheme routing pattern (D = DVE-direct fp32, A = ACT->DVE bf16)
PAT1 = ['D', 'A', 'A', 'A'] * 4
PAT2 = PAT1 + PAT1


def build(ni=NI, nj=NJ, gw=GW, reps=1, scheme="hybrid", unroll=1):
    if scheme == "pf":
        return build_pf(ni, nj, min(gw, 1024), reps)
    if scheme == "v2":
        return build_v2(ni, nj, gw, reps)
    if scheme == "v4":
        return build_v4(ni, nj, min(gw, 2048), reps, unroll=unroll)
    if scheme == "v5":
        return build_v5(ni, nj, min(gw, 2048), reps, unroll=unroll)
    n_ic = ni // 128
    n_jg = nj // gw

    nc = bacc.Bacc("TRN2", target_bir_lowering=False, debug=False,
                   enable_asserts=False, num_devices=NCORES)
    f32 = mybir.dt.float32
    bf16 = mybir.dt.bfloat16

    xT_d = nc.dram_tensor("xT", [128, ni], bf16, kind="ExternalInput")
    m2yT_d = nc.dram_tensor("m2yT", [128, nj], bf16, kind="ExternalInput")
    x2c_d = nc.dram_tensor("x2c", [128, n_ic], f32, kind="ExternalInput")
    y2bc_d = nc.dram_tensor("y2bc", [128, nj], bf16, kind="ExternalInput")
    colB_d = nc.dram_tensor("colB", [128, nj], bf16, kind="ExternalOutput")
    rowR_d = nc.dram_tensor("rowR", [128, n_ic * n_jg], f32, kind="ExternalOutput")

    with tile.TileContext(nc) as tc:
        with (
            tc.tile_pool(name="persist", bufs=1) as persist,
            tc.tile_pool(name="psum", bufs=2, space="PSUM") as psum_pool,
            tc.tile_pool(name="u", bufs=3) as upool,
        ):
            xT = persist.tile([128, ni], bf16, name="xT")
            m2yT = persist.tile([128, nj], bf16, name="m2yT")
            x2c = persist.tile([128, n_ic], f32, name="x2c")
            y2bc = persist.tile([128, nj], bf16, name="y2bc")
            colB = persist.tile([128, nj], bf16, name="colB")
            rowR = persist.tile([128, n_ic * n_jg], f32, name="rowR")

            nc.sync.dma_start(out=xT[:, :], in_=xT_d[:, :])
            ck = min(2048, nj)
            for c0 in range(0, nj, ck):
                nc.sync.dma_start(out=m2yT[:, c0:c0 + ck], in_=m2yT_d[:, c0:c0 + ck])
                nc.sync.dma_start(out=y2bc[:, c0:c0 + ck], in_=y2bc_d[:, c0:c0 + ck])
            nc.sync.dma_start(out=x2c[:, :], in_=x2c_d[:, :])
            nc.vector.memset(colB[:, :], BIG)

            def emit_body():
                for ic in range(n_ic):
                    for jg in range(n_jg):
                        sl = slice(jg * gw, (jg + 1) * gw)
                        psum = psum_pool.tile([128, gw], f32, tag="ps", name="ps")
                        for q in range(gw // MMW):
                            j0 = jg * gw + q * MMW
                            nc.tensor.matmul(
                                psum[:, q * MMW:(q + 1) * MMW],
                                xT[:, ic * 128:(ic + 1) * 128],
                                m2yT[:, j0:j0 + MMW])
                        u = upool.tile([128, gw], bf16, tag="u", name="u")
                        nc.vector.tensor_tensor(
                            u[:, :], psum[:, :], y2bc[:, sl], AluOp.add)
                        k = ic * n_jg + jg
                        nc.vector.tensor_reduce(
                            rowR[:, k:k + 1], u[:, :],
                            mybir.AxisListType.X, AluOp.min)
                        nc.vector.scalar_tensor_tensor(
                            colB[:, sl], u[:, :], x2c[:, ic:ic + 1],
                            colB[:, sl], AluOp.add, AluOp.min)

            if reps > 1:
                with tc.For_i(0, reps, 1,
                              hint_engines=(mybir.EngineType.PE,
                                            mybir.EngineType.DVE)):
                    emit_body()
            else:
                emit_body()

            for c0 in range(0, nj, ck):
                nc.sync.dma_start(out=colB_d[:, c0:c0 + ck], in_=colB[:, c0:c0 + ck])
            nc.sync.dma_start(out=rowR_d[:, :], in_=rowR[:, :])

    nc.compile()
    return nc


def build_v2(ni=NI, nj=NJ, gw=2048, reps=1):
    """Fused scheme: per [128 x gw] psum group exactly TWO DVE ops.

    tensor_tensor_reduce: u = psum + y2bc (bf16, dead store);
                          rowR[:, k] = min_j u            (row path)
    scalar_tensor_tensor: colB = min(colB, psum + x2_i)   (col path;
                          y2_j commutes with min over i -> host adds it)
    """
    n_ic = ni // 128
    n_jg = nj // gw
    psum_bufs = 2 if gw <= 2048 else 1

    nc = bacc.Bacc("TRN2", target_bir_lowering=False, debug=False,
                   enable_asserts=False, num_devices=NCORES)
    f32 = mybir.dt.float32
    bf16 = mybir.dt.bfloat16

    xT_d = nc.dram_tensor("xT", [128, ni], bf16, kind="ExternalInput")
    m2yT_d = nc.dram_tensor("m2yT", [128, nj], bf16, kind="ExternalInput")
    x2c_d = nc.dram_tensor("x2c", [128, n_ic], f32, kind="ExternalInput")
    y2bc_d = nc.dram_tensor("y2bc", [128, nj], bf16, kind="ExternalInput")
    colB_d = nc.dram_tensor("colB", [128, nj], f32, kind="ExternalOutput")
    rowR_d = nc.dram_tensor("rowR", [128, n_ic * n_jg], f32, kind="ExternalOutput")

    with tile.TileContext(nc) as tc:
        with (
            tc.tile_pool(name="persist", bufs=1) as persist,
            tc.tile_pool(name="psum", bufs=psum_bufs, space="PSUM") as psum_pool,
            tc.tile_pool(name="u", bufs=3) as upool,
        ):
            xT = persist.tile([128, ni], bf16, name="xT")
            m2yT = persist.tile([128, nj], bf16, name="m2yT")
            x2c = persist.tile([128, n_ic], f32, name="x2c")
            y2bc = persist.tile([128, nj], bf16, name="y2bc")
            colB = persist.tile([128, nj], f32, name="colB")
            rowR = persist.tile([128, n_ic * n_jg], f32, name="rowR")

            nc.sync.dma_start(out=xT[:, :], in_=xT_d[:, :])
            ck = min(2048, nj)
            for c0 in range(0, nj, ck):
                nc.sync.dma_start(out=m2yT[:, c0:c0 + ck], in_=m2yT_d[:, c0:c0 + ck])
                nc.sync.dma_start(out=y2bc[:, c0:c0 + ck], in_=y2bc_d[:, c0:c0 + ck])
            nc.sync.dma_start(out=x2c[:, :], in_=x2c_d[:, :])
            nc.vector.memset(colB[:, :], BIG)

            def emit_body():
                for ic in range(n_ic):
                    for jg in range(n_jg):
                        sl = slice(jg * gw, (jg + 1) * gw)
                        psum = psum_pool.tile([128, gw], f32, tag="ps", name="ps")
                        for q in range(gw // MMW):
                            j0 = jg * gw + q * MMW
                            nc.tensor.matmul(
                                psum[:, q * MMW:(q + 1) * MMW],
                                xT[:, ic * 128:(ic + 1) * 128],
                                m2yT[:, j0:j0 + MMW])
                        u = upool.tile([128, gw], bf16, tag="u", name="u")
                        k = ic * n_jg + jg
                        nc.vector.tensor_tensor_reduce(
                            out=u[:, :], in0=psum[:, :], in1=y2bc[:, sl],
                            scale=1.0, scalar=BIG,
                            op0=AluOp.add, op1=AluOp.min,
                            accum_out=rowR[:, k:k + 1])
                        nc.vector.scalar_tensor_tensor(
                            colB[:, sl], psum[:, :], x2c[:, ic:ic + 1],
                            colB[:, sl], AluOp.add, AluOp.min)

            if reps > 1:
                with tc.For_i(0, reps, 1,
                              hint_engines=(mybir.EngineType.PE,
                                            mybir.EngineType.DVE)):
                    emit_body()
            else:
                emit_body()

            for c0 in range(0, nj, ck):
                nc.sync.dma_start(out=colB_d[:, c0:c0 + ck], in_=colB[:, c0:c0 + ck])
            nc.sync.dma_start(out=rowR_d[:, :], in_=rowR[:, :])

    nc.compile()
    return nc


def build_v4(ni=NI, nj=NJ, gw=2048, reps=1, unroll=1):
    """3-engine pipeline, minimal DVE work.

    PE:  psum = y2_j - 2 x_i.y_j   (main matmul + K=1 ones-row accumulate
         matmul that broadcasts y2 along partitions)
    ACT: u16 = bf16(psum + x2_i)   (per-partition bias; u16 = full P)
    DVE: rowR[:, ic] = min_j u16   (tensor_reduce, FD = nj)
         colB = min(colB, u16)     (tensor_tensor min, bf16 2x)
    Host: clips + means; no bias corrections needed (P is complete).
    """
    n_ic = ni // 128
    n_jg = nj // gw

    nc = bacc.Bacc("TRN2", target_bir_lowering=False, debug=False,
                   enable_asserts=False, num_devices=NCORES)
    f32 = mybir.dt.float32
    bf16 = mybir.dt.bfloat16

    xT_d = nc.dram_tensor("xT", [128, ni], bf16, kind="ExternalInput")
    m2yT_d = nc.dram_tensor("m2yT", [128, nj], bf16, kind="ExternalInput")
    x2c_d = nc.dram_tensor("x2c", [128, n_ic], f32, kind="ExternalInput")
    y2r_d = nc.dram_tensor("y2r", [1, nj], bf16, kind="ExternalInput")
    colB_d = nc.dram_tensor("colB", [128, nj], bf16, kind="ExternalOutput")
    rowR_d = nc.dram_tensor("rowR", [128, n_ic], f32, kind="ExternalOutput")

    with tile.TileContext(nc) as tc:
        with (
            tc.tile_pool(name="persist", bufs=1) as persist,
            tc.tile_pool(name="psum", bufs=2, space="PSUM") as psum_pool,
            tc.tile_pool(name="u", bufs=2) as upool,
        ):
            xT = persist.tile([128, ni], bf16, name="xT")
            m2yT = persist.tile([128, nj], bf16, name="m2yT")
            x2c = persist.tile([128, n_ic], f32, name="x2c")
            y2r = persist.tile([1, nj], bf16, name="y2r")
            ones = persist.tile([1, 128], bf16, name="ones")
            colB = persist.tile([128, nj], bf16, name="colB")
            rowR = persist.tile([128, n_ic], f32, name="rowR")

            nc.sync.dma_start(out=xT[:, :], in_=xT_d[:, :])
            nc.sync.dma_start(out=m2yT[:, :], in_=m2yT_d[:, :])
            nc.sync.dma_start(out=x2c[:, :], in_=x2c_d[:, :])
            nc.sync.dma_start(out=y2r[:, :], in_=y2r_d[:, :])
            nc.vector.memset(ones[:, :], 1.0)
            nc.vector.memset(colB[:, :], BIG)

            def emit_body():
                for ic in range(n_ic):
                    u16 = upool.tile([128, nj], bf16, tag="u", name="u")
                    for jg in range(n_jg):
                        sl = slice(jg * gw, (jg + 1) * gw)
                        psum = psum_pool.tile([128, gw], f32, tag="ps",
                                              name="ps")
                        for q in range(gw // MMW):
                            j0 = jg * gw + q * MMW
                            qs = slice(q * MMW, (q + 1) * MMW)
                            nc.tensor.matmul(
                                psum[:, qs],
                                xT[:, ic * 128:(ic + 1) * 128],
                                m2yT[:, j0:j0 + MMW],
                                start=True, stop=False)
                            nc.tensor.matmul(
                                psum[:, qs], ones[:, :],
                                y2r[:, j0:j0 + MMW],
                                start=False, stop=True)
                        nc.scalar.activation(
                            u16[:, sl], psum[:, :], ActFn.Identity,
                            bias=x2c[:, ic:ic + 1], scale=1.0)
                    nc.vector.tensor_reduce(
                        rowR[:, ic:ic + 1], u16[:, :],
                        mybir.AxisListType.X, AluOp.min)
                    nc.vector.tensor_tensor(
                        colB[:, :], u16[:, :], colB[:, :], AluOp.min)

            if reps > 1:
                with tc.For_i(0, reps, 1,
                              hint_engines=(mybir.EngineType.PE,
                                            mybir.EngineType.DVE,
                                            mybir.EngineType.Activation)):
                    emit_body()
            else:
                for _ in range(unroll):
                    emit_body()

            nc.sync.dma_start(out=colB_d[:, :], in_=colB[:, :])
            nc.sync.dma_start(out=rowR_d[:, :], in_=rowR[:, :])

    nc.compile()
    return nc


def build_v5(ni=NI, nj=NJ, gw=2048, reps=1, unroll=1):
    """Minimal-instruction 3-engine pipeline (no bias matmuls).

    PE:  psum = -2 x_i.y_j                       (8x 512-wide MM per ic)
    ACT: u16 = bf16(psum + x2_i)                 (2 per ic, psum halves)
    DVE per ic (FD = nj):
         v = u16 + y2bc        (TT bf16 2x; v = full P)
         rowR[:, ic] = min_j v (TR)
         colB = min(colB, v)   (TT min; includes x2+y2 -> host just clips)
    """
    n_ic = ni // 128
    n_jg = nj // gw

    nc = bacc.Bacc("TRN2", target_bir_lowering=False, debug=False,
                   enable_asserts=False, num_devices=NCORES)
    f32 = mybir.dt.float32
    bf16 = mybir.dt.bfloat16

    xT_d = nc.dram_tensor("xT", [128, ni], bf16, kind="ExternalInput")
    m2yT_d = nc.dram_tensor("m2yT", [128, nj], bf16, kind="ExternalInput")
    x2c_d = nc.dram_tensor("x2c", [128, n_ic], f32, kind="ExternalInput")
    y2bc_d = nc.dram_tensor("y2bc", [128, nj], bf16, kind="ExternalInput")
    colB_d = nc.dram_tensor("colB", [128, nj], bf16, kind="ExternalOutput")
    rowR_d = nc.dram_tensor("rowR", [128, n_ic], f32, kind="ExternalOutput")

    with tile.TileContext(nc) as tc:
        with (
            tc.tile_pool(name="persist", bufs=1) as persist,
            tc.tile_pool(name="psum", bufs=2, space="PSUM") as psum_pool,
            tc.tile_pool(name="u", bufs=2) as upool,
            tc.tile_pool(name="v", bufs=2) as vpool,
        ):
            xT = persist.tile([128, ni], bf16, name="xT")
            m2yT = persist.tile([128, nj], bf16, name="m2yT")
            x2c = persist.tile([128, n_ic], f32, name="x2c")
            y2bc = persist.tile([128, nj], bf16, name="y2bc")
            colB = persist.tile([128, nj], bf16, name="colB")
            rowR = persist.tile([128, n_ic], f32, name="rowR")

            nc.sync.dma_start(out=xT[:, :], in_=xT_d[:, :])
            nc.sync.dma_start(out=m2yT[:, :], in_=m2yT_d[:, :])
            nc.sync.dma_start(out=x2c[:, :], in_=x2c_d[:, :])
            nc.sync.dma_start(out=y2bc[:, :], in_=y2bc_d[:, :])
            nc.vector.memset(colB[:, :], BIG)

            def emit_body():
                for ic in range(n_ic):
                    u16 = upool.tile([128, nj], bf16, tag="u", name="u")
                    for jg in range(n_jg):
                        sl = slice(jg * gw, (jg + 1) * gw)
                        psum = psum_pool.tile([128, gw], f32, tag="ps",
                                              name="ps")
                        for q in range(gw // MMW):
                            j0 = jg * gw + q * MMW
                            nc.tensor.matmul(
                                psum[:, q * MMW:(q + 1) * MMW],
                                xT[:, ic * 128:(ic + 1) * 128],
                                m2yT[:, j0:j0 + MMW])
                        nc.scalar.activation(
                            u16[:, sl], psum[:, :], ActFn.Identity,
                            bias=x2c[:, ic:ic + 1], scale=1.0)
                    v = vpool.tile([128, nj], bf16, tag="v", name="v")
                    nc.vector.tensor_tensor(
                        v[:, :], u16[:, :], y2bc[:, :], AluOp.add)
                    nc.vector.tensor_reduce(
                        rowR[:, ic:ic + 1], v[:, :],
                        mybir.AxisListType.X, AluOp.min)
                    nc.vector.tensor_tensor(
                        colB[:, :], v[:, :], colB[:, :], AluOp.min)

            if reps > 1:
                with tc.For_i(0, reps, 1,
                              hint_engines=(mybir.EngineType.PE,
                                            mybir.EngineType.DVE,
                                            mybir.EngineType.Activation)):
                    emit_body()
            else:
                for _ in range(unroll):
                    emit_body()

            nc.sync.dma_start(out=colB_d[:, :], in_=colB[:, :])
            nc.sync.dma_start(out=rowR_d[:, :], in_=rowR[:, :])

    nc.compile()
    return nc


def build_pf(ni, nj, gw, reps):
    """Two-orientation scheme with DVE/ACT split (fallback / A-B testing)."""
    n_ic = ni // 128
    n_jc = nj // 128
    pat1 = PAT1[:n_ic]
    pat2 = PAT2[:n_jc]
    paths = set(pat1) | set(pat2)

    nc = bacc.Bacc("TRN2", target_bir_lowering=False, debug=False,
                   enable_asserts=False, num_devices=NCORES)
    f32 = mybir.dt.float32
    bf16 = mybir.dt.bfloat16

    xT_d = nc.dram_tensor("xT", [128, ni], bf16, kind="ExternalInput")
    m2yT_d = nc.dram_tensor("m2yT", [128, nj], bf16, kind="ExternalInput")
    x2c_d = nc.dram_tensor("x2c", [128, n_ic], f32, kind="ExternalInput")
    y2c_d = nc.dram_tensor("y2c", [128, n_jc], f32, kind="ExternalInput")
    col_d, row_d = {}, {}
    for p in sorted(paths):
        dt = f32 if p == 'D' else bf16
        col_d[p] = nc.dram_tensor("col" + p, [128, nj], dt, kind="ExternalOutput")
        row_d[p] = nc.dram_tensor("row" + p, [128, ni], dt, kind="ExternalOutput")

    with tile.TileContext(nc) as tc:
        with (
            tc.tile_pool(name="persist", bufs=1) as persist,
            tc.tile_pool(name="psum", bufs=4, space="PSUM") as psum_pool,
            tc.tile_pool(name="u", bufs=6) as upool,
        ):
            xT = persist.tile([128, ni], bf16, name="xT")
            m2yT = persist.tile([128, nj], bf16, name="m2yT")
            x2c = persist.tile([128, n_ic], f32, name="x2c")
            y2c = persist.tile([128, n_jc], f32, name="y2c")
            col_s = {p: persist.tile([128, nj], f32 if p == 'D' else bf16,
                                     name="col" + p, tag="col" + p)
                     for p in sorted(paths)}
            row_s = {p: persist.tile([128, ni], f32 if p == 'D' else bf16,
                                     name="row" + p, tag="row" + p)
                     for p in sorted(paths)}

            ck = min(1024, ni, nj)
            for c0 in range(0, ni, ck):
                nc.sync.dma_start(out=xT[:, c0:c0 + ck], in_=xT_d[:, c0:c0 + ck])
            for c0 in range(0, nj, ck):
                nc.sync.dma_start(out=m2yT[:, c0:c0 + ck], in_=m2yT_d[:, c0:c0 + ck])
            nc.sync.dma_start(out=x2c[:, :], in_=x2c_d[:, :])
            nc.sync.dma_start(out=y2c[:, :], in_=y2c_d[:, :])

            def consume(path, psum, bias, accs, sl, first):
                acc = accs[path]
                if path == 'D':
                    if first:
                        nc.vector.tensor_scalar(
                            acc[:, sl], psum[:, :], bias, None, AluOp.add)
                    else:
                        nc.vector.scalar_tensor_tensor(
                            acc[:, sl], psum[:, :], bias, acc[:, sl],
                            AluOp.add, AluOp.min)
                    return
                u = upool.tile([128, psum.shape[1]], bf16, name="u", tag="u")
                nc.scalar.activation(u[:, :], psum[:, :], ActFn.Identity,
                                     bias=bias, scale=1.0)
                if first:
                    nc.vector.tensor_copy(acc[:, sl], u[:, :])
                else:
                    nc.vector.tensor_tensor(acc[:, sl], u[:, :], acc[:, sl],
                                            AluOp.min)

            def emit_body():
                for jg in range(nj // gw):
                    sl = slice(jg * gw, (jg + 1) * gw)
                    seen = set()
                    for ic in range(n_ic):
                        path = pat1[ic]
                        psum = psum_pool.tile([128, gw], f32, tag="ps", name="ps")
                        for q in range(gw // MMW):
                            j0 = jg * gw + q * MMW
                            nc.tensor.matmul(
                                psum[:, q * MMW:(q + 1) * MMW],
                                xT[:, ic * 128:(ic + 1) * 128],
                                m2yT[:, j0:j0 + MMW])
                        consume(path, psum, x2c[:, ic:ic + 1], col_s, sl,
                                path not in seen)
                        seen.add(path)
                gw2 = min(gw, ni)
                for ig in range(ni // gw2):
                    sl = slice(ig * gw2, (ig + 1) * gw2)
                    seen = set()
                    for jc in range(n_jc):
                        path = pat2[jc]
                        psum = psum_pool.tile([128, gw2], f32, tag="ps", name="ps")
                        for q in range(gw2 // MMW):
                            i0 = ig * gw2 + q * MMW
                            nc.tensor.matmul(
                                psum[:, q * MMW:(q + 1) * MMW],
                                m2yT[:, jc * 128:(jc + 1) * 128],
                                xT[:, i0:i0 + MMW])
                        consume(path, psum, y2c[:, jc:jc + 1], row_s, sl,
                                path not in seen)
                        seen.add(path)

            if reps > 1:
                with tc.For_i(0, reps, 1,
                              hint_engines=(mybir.EngineType.PE,
                                            mybir.EngineType.DVE,
                                            mybir.EngineType.Activation)):
                    emit_body()
            else:
                emit_body()

            for p in sorted(paths):
                nc.sync.dma_start(out=col_d[p][:, :], in_=col_s[p][:, :])
                nc.sync.dma_start(out=row_d[p][:, :], in_=row_s[p][:, :])

    nc.compile()
    return nc


def host_prep(x, y, scheme="hybrid"):
    """Per-core input maps. Core c: batch c//2, i-half c%2."""
    x = np.ascontiguousarray(np.asarray(x, F32))
    y = np.ascontiguousarray(np.asarray(y, F32))
    x16 = x.astype(BF16)
    y16 = y.astype(BF16)
    m2y16 = (y16.astype(F32) * -2.0).astype(BF16)          # exact in bf16
    x2 = (x16.astype(F32) ** 2).sum(-1)                    # [B, N]
    y2 = (y16.astype(F32) ** 2).sum(-1)
    in_maps = []
    for c in range(NCORES):
        b, h = divmod(c, 2)
        i0 = h * NI
        m = {
            "xT": np.ascontiguousarray(x16[b, i0:i0 + NI, :].T),
            "m2yT": np.ascontiguousarray(m2y16[b].T),
            "x2c": np.ascontiguousarray(x2[b, i0:i0 + NI].reshape(NI // 128, 128).T),
        }
        if scheme == "v4":
            m["y2r"] = np.ascontiguousarray(y2[b].astype(BF16)[None, :])
        elif scheme in ("hybrid", "v2", "v5"):
            m["y2bc"] = np.ascontiguousarray(
                np.broadcast_to(y2[b].astype(BF16), (128, N)))
        else:
            m["y2c"] = np.ascontiguousarray(y2[b].reshape(N // 128, 128).T)
        in_maps.append(m)
    return in_maps, x2, y2


def combine(results, x2, y2, scheme="hybrid"):
    col_mins = np.empty((B, N), F32)
    row_mins = np.empty((B, N), F32)
    for b in range(B):
        cores = [results[2 * b], results[2 * b + 1]]
        if scheme in ("v4", "v5"):
            col = np.minimum.reduce(
                [r["colB"].astype(F32).min(0) for r in cores])
            col_mins[b] = np.clip(col, 0.0, 100.0)
            for h, r in enumerate(cores):
                row = r["rowR"].T.reshape(-1)          # [NI], i = ic*128+lane
                i0 = h * NI
                row_mins[b, i0:i0 + NI] = np.clip(row, 0.0, 100.0)
        elif scheme == "v2":
            col = np.minimum.reduce([r["colB"].min(0) for r in cores])
            col_mins[b] = np.clip(col + y2[b], 0.0, 100.0)
            for h, r in enumerate(cores):
                rr = r["rowR"]                         # [128, n_ic*n_jg]
                n_jg = rr.shape[1] // (NI // 128)
                rr = rr.reshape(128, NI // 128, n_jg).min(axis=2)
                row = rr.T.reshape(-1)                 # [NI], i = ic*128 + lane
                i0 = h * NI
                row_mins[b, i0:i0 + NI] = np.clip(
                    row + x2[b, i0:i0 + NI], 0.0, 100.0)
        elif scheme == "hybrid":
            col = np.minimum.reduce([r["colB"].astype(F32).min(0) for r in cores])
            col_mins[b] = np.clip(col, 0.0, 100.0)
            for h, r in enumerate(cores):
                rr = r["rowR"]                         # [128, n_ic*n_jg]
                n_jg = N // GW
                rr = rr.reshape(128, NI // 128, n_jg).min(axis=2)
                row = rr.T.reshape(-1)                 # [NI], i = ic*128 + lane
                i0 = h * NI
                row_mins[b, i0:i0 + NI] = np.clip(
                    row + x2[b, i0:i0 + NI], 0.0, 100.0)
        else:
            col = np.minimum.reduce([
                np.minimum.reduce([r[k].astype(F32).min(0)
                                   for k in r if k.startswith("col")])
                for r in cores])
            col_mins[b] = np.clip(col + y2[b], 0.0, 100.0)
            for h, r in enumerate(cores):
                row = np.minimum.reduce([r[k].astype(F32).min(0)
                                         for k in r if k.startswith("row")])
                i0 = h * NI
                row_mins[b, i0:i0 + NI] = np.clip(
                    row + x2[b, i0:i0 + NI], 0.0, 100.0)
    out = (col_mins.mean(dtype=np.float64) + row_mins.mean(dtype=np.float64)) / B
    return np.asarray(out, dtype=F32)


_CACHE = {}
TRACE = False
LAST_RESULTS = None
SCHEME = "hybrid"


def kernel(corr_pred, corr_target):
    global LAST_RESULTS
    key = ("nc", SCHEME)
    if key not in _CACHE:
        _CACHE[key] = build(scheme=SCHEME)
    nc = _CACHE[key]
    in_maps, x2, y2 = host_prep(corr_pred, corr_target, scheme=SCHEME)
    res = run_bass_kernel_spmd(nc, in_maps, core_ids=list(range(NCORES)),
                               trace=TRACE)
    LAST_RESULTS = res
    return combine(res.results, x2, y2, scheme=SCHEME)

